# Initial kernel scaffold
#
"""Trainium2 Bass kernel for a feature-space attention head.

Reference computation (per batch b, with T=4096, E=1024, D=64):
    Q = x @ Wq; K = x @ Wk; V = x @ Wv            # (T,E)@(E,D) -> (T,D)
    R = (K^T @ Q) / sqrt(E)                        # (D,D) feature-space scores
    R = where(strictly_lower, -inf, R); R = softmax(R, axis=-1)
    out = V @ R                                    # (T,D)

Sharding: data-parallel over batch B=8 across the 8 NeuronCores (one batch
per core, no collectives).

Per-core device pipeline (bf16 operands, fp32 PSUM accumulation):
  - SWDGE cast-DMA loads x blocks f32->bf16; PE-transpose to x^T
  - per block pair (pass matmuls clustered to keep the PE clock warm):
    pass1 [Wq/32|Wk] stationary -> [Q^T;K^T]; pass2 Wv -> V^T
  - re-transpose [Q^T;K^T] -> [Q|K] natural, accumulate R += K^T Q in PSUM
    across all T (software-pipelined into the next transpose phase)
  - masked softmax on R (64x64) in fp32, O = V @ P via V^T-stationary
    chunks, per-group output DMA (fp32 out).
"""

import os
import sys

import numpy as np

for _p in ("/opt/trn_rl_repo", "/root/.axon_site/_ro/trn_rl_repo"):
    if os.path.isdir(_p) and _p not in sys.path:
        sys.path.append(_p)

import concourse.bass as bass  # noqa: E402
import concourse.tile as tile  # noqa: E402
from concourse import bacc, mybir  # noqa: E402
from concourse.bass_utils import run_bass_kernel_spmd  # noqa: E402
from concourse.masks import make_identity  # noqa: E402

B, T, E, D = 8, 4096, 1024, 64
N_CORES = 8
TBLK = 512                # t rows per block
NBLK = T // TBLK          # 8 blocks
NSUB = TBLK // 128        # 4 t-subtiles per block
ECH = E // 128            # 8 e-chunks

F32 = mybir.dt.float32
BF16 = mybir.dt.bfloat16
AX = mybir.AxisListType
AF = mybir.ActivationFunctionType

_COMPILED = None


def _build():
    nc = bacc.Bacc("TRN2", target_bir_lowering=False, debug=False,
                   num_devices=N_CORES)
    x = nc.dram_tensor("x", [T, E], F32, kind="ExternalInput").ap()
    wqk = nc.dram_tensor("wqk", [E, 128], F32, kind="ExternalInput").ap()
    wv = nc.dram_tensor("wv", [E, 128], F32, kind="ExternalInput").ap()
    out = nc.dram_tensor("out", [T, D], F32, kind="ExternalOutput").ap()

    # DRAM views: partition-major for DMA
    x_r = x.rearrange("(n p) e -> p n e", p=128)          # [128, 32, 1024]
    wqk_r = wqk.rearrange("(c p) m -> p c m", p=128)      # [128, 8, 128]
    wv_r = wv.rearrange("(c p) m -> p c m", p=128)        # [128, 8, 128]
    out_r = out.rearrange("(c p) d -> p c d", p=128)      # [128, 32, 64]

    with tile.TileContext(nc) as tc:
        with (
            tc.tile_pool(name="const", bufs=1) as constp,
            tc.tile_pool(name="xin", bufs=8) as xinp,
            tc.tile_pool(name="xt", bufs=24) as xtp,
            tc.tile_pool(name="qkt", bufs=4) as qktp,
            tc.tile_pool(name="qkn", bufs=3) as qknp,
            tc.tile_pool(name="vt", bufs=1) as vtp,
            tc.tile_pool(name="small", bufs=1) as smallp,
            tc.tile_pool(name="osb", bufs=4) as osbp,
            tc.tile_pool(name="ps_t", bufs=4, space="PSUM") as ps_t,
            tc.tile_pool(name="ps_qk", bufs=1, space="PSUM") as ps_qk,
            tc.tile_pool(name="ps_v", bufs=1, space="PSUM") as ps_v,
            tc.tile_pool(name="ps_rt", bufs=1, space="PSUM") as ps_rt,
            tc.tile_pool(name="ps_r", bufs=1, space="PSUM") as ps_rp,
        ):
            # issue the first x loads before anything else so HBM traffic
            # starts during the framework preamble
            prefetched = {}
            for blk in range(2):
                xins0 = []
                for h in range(2):
                    xin = xinp.tile([128, 2 * E], BF16, tag="xin")
                    xin3 = xin[:].rearrange("p (n e) -> p n e", n=2)
                    n0 = blk * NSUB + h * 2
                    if blk == 0:
                        # quarter-DMAs: earliest possible first transposes
                        nc.gpsimd.dma_start(xin3[:, 0:1, :],
                                            x_r[:, n0:n0 + 1, :])
                        nc.gpsimd.dma_start(xin3[:, 1:2, :],
                                            x_r[:, n0 + 1:n0 + 2, :])
                    else:
                        nc.gpsimd.dma_start(xin3, x_r[:, n0:n0 + 2, :])
                    xins0.append(xin3)
                prefetched[blk] = xins0

            ident16 = constp.tile([128, 128], BF16)
            make_identity(nc, ident16[:])
            # additive mask: 0 where i<=j, -1e30 strictly below the diagonal
            mask_sb = constp.tile([64, 64], F32)
            nc.gpsimd.memset(mask_sb[:], 0.0)
            nc.gpsimd.affine_select(
                out=mask_sb[:], in_=mask_sb[:],
                compare_op=mybir.AluOpType.is_ge,
                fill=-1e30, base=0, pattern=[[1, 64]], channel_multiplier=-1,
            )

            # weights: HWDGE f32 load (keeps the SWDGE Q7 free for x loads),
            # then one-time DVE casts to bf16
            wqk_f32 = constp.tile([128, ECH * 128], F32)
            wv_f32 = constp.tile([128, ECH * 128], F32)
            nc.sync.dma_start(
                wqk_f32[:].rearrange("p (c m) -> p c m", c=ECH), wqk_r[:])
            nc.sync.dma_start(
                wv_f32[:].rearrange("p (c m) -> p c m", c=ECH), wv_r[:])
            wqk_sb = constp.tile([128, ECH * 128], BF16)
            wv_sb = constp.tile([128, ECH * 128], BF16)
            nc.vector.tensor_copy(wqk_sb[:], wqk_f32[:])
            nc.vector.tensor_copy(wv_sb[:], wv_f32[:])

            vT = vtp.tile([64, T], BF16)          # persistent V^T
            ps_R = ps_rp.tile([64, 64], F32)      # persistent R accumulator

            pending_retr = []    # [(qkT_tile, blk)] to emit during transposes
            pending_pass = []    # [(xts, blk)] pass matmuls deferred to pair end

            def emit_retranspose_r(qkT, blk, first, last):
                prt = ps_rt.tile([128, TBLK], BF16)
                for s in range(NSUB):
                    nc.tensor.transpose(
                        prt[:, s * 128:(s + 1) * 128],
                        qkT[:, s * 128:(s + 1) * 128],
                        ident16[:],
                    )
                qkn = qknp.tile([128, TBLK], BF16)
                nc.vector.tensor_copy(qkn[:], prt[:])
                for s in range(NSUB):
                    nc.tensor.matmul(
                        ps_R[:],
                        qkn[:, s * 128 + 64:(s + 1) * 128],   # K chunk [128t, 64]
                        qkn[:, s * 128:s * 128 + 64],         # Q chunk [128t, 64]
                        start=(first and s == 0),
                        stop=(last and s == NSUB - 1),
                    )

            def emit_passes(xts, blk):
                pqk = ps_qk.tile([128, TBLK], F32)
                for c in range(ECH):
                    nc.tensor.matmul(
                        pqk[:], wqk_sb[:, c * 128:(c + 1) * 128], xts[c],
                        start=(c == 0), stop=(c == ECH - 1),
                    )
                qkT = qktp.tile([128, TBLK], BF16)
                nc.scalar.activation(qkT[:], pqk[:], AF.Copy)
                pending_retr.append((qkT, blk))

                pv = ps_v.tile([64, TBLK], F32)
                for c in range(ECH):
                    nc.tensor.matmul(
                        pv[:], wv_sb[:, c * 128:c * 128 + D], xts[c],
                        start=(c == 0), stop=(c == ECH - 1),
                    )
                nc.vector.tensor_copy(vT[:, blk * TBLK:(blk + 1) * TBLK], pv[:])

            for blk in range(NBLK):
                # ---- load x block (cast to bf16) as two half-block DMAs ----
                if blk in prefetched:
                    xins = prefetched[blk]
                else:
                    xins = []
                    for h in range(2):
                        xin = xinp.tile([128, 2 * E], BF16, tag="xin")
                        xin3 = xin[:].rearrange("p (n e) -> p n e", n=2)
                        n0 = blk * NSUB + h * 2
                        nc.gpsimd.dma_start(xin3, x_r[:, n0:n0 + 2, :])
                        xins.append(xin3)

                # ---- transpose x -> x^T (bf16): 2 e-chunks per PSUM bank ----
                xts = []
                for j in range(ECH // 2):
                    tp = ps_t.tile([128, 2 * TBLK], BF16, tag="ps_t")
                    for half in range(2):
                        c = 2 * j + half
                        for s in range(NSUB):
                            nc.tensor.transpose(
                                tp[:, half * TBLK + s * 128:
                                   half * TBLK + (s + 1) * 128],
                                xins[s // 2][:, s % 2, c * 128:(c + 1) * 128],
                                ident16[:],
                            )
                    xt_j = xtp.tile([128, 2 * TBLK], BF16)
                    if j % 2 == 0:
                        nc.vector.tensor_copy(xt_j[:], tp[:])
                    else:
                        nc.scalar.activation(xt_j[:], tp[:], AF.Copy)
                    xts.extend([xt_j[:, :TBLK], xt_j[:, TBLK:]])
                    if j == 1 and pending_retr:
                        for qkT_p, blk_p in pending_retr:
                            emit_retranspose_r(qkT_p, blk_p, blk_p == 0, False)
                        pending_retr.clear()

                pending_pass.append((xts, blk))
                if blk in (0, 1, 2, 4, 6, 7):
                    # clustered pass matmuls: dense real-MM burst warms the PE
                    for xts_p, blk_p in pending_pass:
                        emit_passes(xts_p, blk_p)
                    pending_pass.clear()

            pending_retr.reverse()
            for i, (qkT_p, blk_p) in enumerate(pending_retr):
                emit_retranspose_r(qkT_p, blk_p, False,
                                   i == len(pending_retr) - 1)
            pending_retr.clear()

            # ---- softmax on R (64x64): fused mask-add from PSUM ----
            r_sb = smallp.tile([64, 64], F32)
            nc.vector.tensor_add(r_sb[:], ps_R[:], mask_sb[:])
            negmax = smallp.tile([64, 1], F32)
            nc.vector.reduce_max(negmax[:], r_sb[:], axis=AX.X, negate=True)
            p_exp = smallp.tile([64, 64], F32)
            rowsum = smallp.tile([64, 1], F32)
            nc.scalar.activation(p_exp[:], r_sb[:], AF.Exp,
                                 bias=negmax[:], scale=1.0, accum_out=rowsum[:])
            rinv = smallp.tile([64, 1], F32)
            nc.vector.reciprocal(rinv[:], rowsum[:])
            p_r = smallp.tile([64, 64], BF16)
            nc.vector.tensor_scalar_mul(p_r[:], p_exp[:], rinv[:])

            # ---- O = V @ P : lhsT = V^T chunks, rhs = P; DMA out per group ----
            for g in range(4):
                po = ps_t.tile([128, 512], F32, tag="ps_t")
                for k in range(8):
                    c = g * 8 + k
                    nc.tensor.matmul(
                        po[:, k * D:(k + 1) * D],
                        vT[:, c * 128:(c + 1) * 128], p_r[:],
                        start=True, stop=True,
                    )
                o_sb = osbp.tile([128, 512], F32)
                if g % 2 == 0:
                    nc.scalar.activation(o_sb[:], po[:], AF.Copy)
                else:
                    nc.vector.tensor_copy(o_sb[:], po[:])
                nc.sync.dma_start(
                    out_r[:, g * 8:(g + 1) * 8, :],
                    o_sb[:].rearrange("p (c d) -> p c d", c=8),
                )

    nc.compile()
    return nc


def kernel(x, Wq, Wk, Wv):
    global _COMPILED
    if _COMPILED is None:
        _COMPILED = _build()
    nc = _COMPILED

    x = np.ascontiguousarray(np.asarray(x), dtype=np.float32)
    # fold the 1/sqrt(E) score scale into Wq (1/32 is exact in f32)
    wqk_h = np.ascontiguousarray(
        np.concatenate([np.asarray(Wq) * (1.0 / 32.0), np.asarray(Wk)], axis=1),
        dtype=np.float32,
    )
    wv_np = np.asarray(Wv)
    wv_h = np.ascontiguousarray(
        np.concatenate([wv_np, wv_np], axis=1), dtype=np.float32)

    in_maps = [
        {"x": np.ascontiguousarray(x[b]), "wqk": wqk_h, "wv": wv_h}
        for b in range(B)
    ]
    res = run_bass_kernel_spmd(nc, in_maps, list(range(N_CORES)))
    return np.stack([res.results[b]["out"] for b in range(B)], axis=0)



# revision 1
# speedup vs baseline: 1.1153x; 1.1153x over previous
"""Trainium2 Bass kernel for a feature-space attention head.

Reference computation (per batch b, with T=4096, E=1024, D=64):
    Q = x @ Wq; K = x @ Wk; V = x @ Wv            # (T,E)@(E,D) -> (T,D)
    R = (K^T @ Q) / sqrt(E)                        # (D,D) feature-space scores
    R = where(strictly_lower, -inf, R); R = softmax(R, axis=-1)
    out = V @ R                                    # (T,D)

Sharding: data-parallel over batch B=8 across the 8 NeuronCores (one batch
per core, no collectives).

Per-core device pipeline (bf16 operands, fp32 PSUM accumulation):
  - SWDGE cast-DMA loads x blocks f32->bf16; PE-transpose to x^T
  - per block pair (pass matmuls clustered to keep the PE clock warm):
    pass1 [Wq/32|Wk] stationary -> [Q^T;K^T]; pass2 Wv -> V^T
  - re-transpose [Q^T;K^T] -> [Q|K] natural, accumulate R += K^T Q in PSUM
    across all T (software-pipelined into the next transpose phase)
  - masked softmax on R (64x64) in fp32, O = V @ P via V^T-stationary
    chunks, per-group output DMA (fp32 out).
"""

import os
import sys

import numpy as np

for _p in ("/opt/trn_rl_repo", "/root/.axon_site/_ro/trn_rl_repo"):
    if os.path.isdir(_p) and _p not in sys.path:
        sys.path.append(_p)

import concourse.bass as bass  # noqa: E402
import concourse.tile as tile  # noqa: E402
from concourse import bacc, mybir  # noqa: E402
from concourse.bass_utils import run_bass_kernel_spmd  # noqa: E402
from concourse.masks import make_identity  # noqa: E402

B, T, E, D = 8, 4096, 1024, 64
N_CORES = 8
TBLK = 512                # t rows per block
NBLK = T // TBLK          # 8 blocks
NSUB = TBLK // 128        # 4 t-subtiles per block
ECH = E // 128            # 8 e-chunks

F32 = mybir.dt.float32
BF16 = mybir.dt.bfloat16
AX = mybir.AxisListType
AF = mybir.ActivationFunctionType

_COMPILED = None


def _build():
    nc = bacc.Bacc("TRN2", target_bir_lowering=False, debug=False,
                   num_devices=N_CORES)
    x = nc.dram_tensor("x", [T, E], F32, kind="ExternalInput").ap()
    wqk = nc.dram_tensor("wqk", [E, 128], F32, kind="ExternalInput").ap()
    wv = nc.dram_tensor("wv", [E, 128], F32, kind="ExternalInput").ap()
    out = nc.dram_tensor("out", [T, D], F32, kind="ExternalOutput").ap()

    # DRAM views: partition-major for DMA
    x_r = x.rearrange("(n p) e -> p n e", p=128)          # [128, 32, 1024]
    wqk_r = wqk.rearrange("(c p) m -> p c m", p=128)      # [128, 8, 128]
    wv_r = wv.rearrange("(c p) m -> p c m", p=128)        # [128, 8, 128]
    out_r = out.rearrange("(c p) d -> p c d", p=128)      # [128, 32, 64]

    with tile.TileContext(nc) as tc:
        with (
            tc.tile_pool(name="const", bufs=1) as constp,
            tc.tile_pool(name="xin", bufs=8) as xinp,
            tc.tile_pool(name="xt", bufs=24) as xtp,
            tc.tile_pool(name="qkt", bufs=4) as qktp,
            tc.tile_pool(name="qkn", bufs=3) as qknp,
            tc.tile_pool(name="vt", bufs=1) as vtp,
            tc.tile_pool(name="small", bufs=1) as smallp,
            tc.tile_pool(name="osb", bufs=4) as osbp,
            tc.tile_pool(name="ps_t", bufs=4, space="PSUM") as ps_t,
            tc.tile_pool(name="ps_qk", bufs=1, space="PSUM") as ps_qk,
            tc.tile_pool(name="ps_v", bufs=1, space="PSUM") as ps_v,
            tc.tile_pool(name="ps_rt", bufs=1, space="PSUM") as ps_rt,
            tc.tile_pool(name="ps_r", bufs=1, space="PSUM") as ps_rp,
        ):
            # issue the first x loads before anything else so HBM traffic
            # starts during the framework preamble
            prefetched = {}
            for blk in range(2):
                xins0 = []
                for h in range(2):
                    xin = xinp.tile([128, 2 * E], BF16, tag="xin")
                    xin3 = xin[:].rearrange("p (n e) -> p n e", n=2)
                    n0 = blk * NSUB + h * 2
                    if blk == 0:
                        # quarter-DMAs: earliest possible first transposes
                        nc.gpsimd.dma_start(xin3[:, 0:1, :],
                                            x_r[:, n0:n0 + 1, :])
                        nc.gpsimd.dma_start(xin3[:, 1:2, :],
                                            x_r[:, n0 + 1:n0 + 2, :])
                    else:
                        nc.gpsimd.dma_start(xin3, x_r[:, n0:n0 + 2, :])
                    xins0.append(xin3)
                prefetched[blk] = xins0

            ident16 = constp.tile([128, 128], BF16)
            make_identity(nc, ident16[:])
            # additive mask: 0 where i<=j, -1e30 strictly below the diagonal
            mask_sb = constp.tile([64, 64], F32)
            nc.gpsimd.memset(mask_sb[:], 0.0)
            nc.gpsimd.affine_select(
                out=mask_sb[:], in_=mask_sb[:],
                compare_op=mybir.AluOpType.is_ge,
                fill=-1e30, base=0, pattern=[[1, 64]], channel_multiplier=-1,
            )

            # weights: HWDGE f32 load (keeps the SWDGE Q7 free for x loads),
            # then one-time DVE casts to bf16
            wqk_f32 = constp.tile([128, ECH * 128], F32)
            wv_f32 = constp.tile([128, ECH * 128], F32)
            nc.sync.dma_start(
                wqk_f32[:].rearrange("p (c m) -> p c m", c=ECH), wqk_r[:])
            nc.sync.dma_start(
                wv_f32[:].rearrange("p (c m) -> p c m", c=ECH), wv_r[:])
            wqk_sb = constp.tile([128, ECH * 128], BF16)
            wv_sb = constp.tile([128, ECH * 128], BF16)
            nc.vector.tensor_copy(wqk_sb[:], wqk_f32[:])
            nc.vector.tensor_copy(wv_sb[:], wv_f32[:])

            vT = vtp.tile([64, T], BF16)          # persistent V^T
            ps_R = ps_rp.tile([64, 64], F32)      # persistent R accumulator

            pending_retr = []    # [(qkT_tile, blk)] to emit during transposes
            pending_pass = []    # [(xts, blk)] pass matmuls deferred to pair end

            def emit_retranspose_r(qkT, blk, first, last):
                prt = ps_rt.tile([128, TBLK], BF16)
                for s in range(NSUB):
                    nc.tensor.transpose(
                        prt[:, s * 128:(s + 1) * 128],
                        qkT[:, s * 128:(s + 1) * 128],
                        ident16[:],
                    )
                qkn = qknp.tile([128, TBLK], BF16)
                nc.vector.tensor_copy(qkn[:], prt[:])
                for s in range(NSUB):
                    nc.tensor.matmul(
                        ps_R[:],
                        qkn[:, s * 128 + 64:(s + 1) * 128],   # K chunk [128t, 64]
                        qkn[:, s * 128:s * 128 + 64],         # Q chunk [128t, 64]
                        start=(first and s == 0),
                        stop=(last and s == NSUB - 1),
                    )

            def emit_passes(xts, blk):
                pqk = ps_qk.tile([128, TBLK], F32)
                for c in range(ECH):
                    nc.tensor.matmul(
                        pqk[:], wqk_sb[:, c * 128:(c + 1) * 128], xts[c],
                        start=(c == 0), stop=(c == ECH - 1),
                    )
                qkT = qktp.tile([128, TBLK], BF16)
                nc.scalar.activation(qkT[:], pqk[:], AF.Copy)
                pending_retr.append((qkT, blk))

                pv = ps_v.tile([64, TBLK], F32)
                for c in range(ECH):
                    nc.tensor.matmul(
                        pv[:], wv_sb[:, c * 128:c * 128 + D], xts[c],
                        start=(c == 0), stop=(c == ECH - 1),
                    )
                nc.vector.tensor_copy(vT[:, blk * TBLK:(blk + 1) * TBLK], pv[:])

            for blk in range(NBLK):
                # ---- load x block (cast to bf16) as two half-block DMAs ----
                if blk in prefetched:
                    xins = prefetched[blk]
                else:
                    xins = []
                    for h in range(2):
                        xin = xinp.tile([128, 2 * E], BF16, tag="xin")
                        xin3 = xin[:].rearrange("p (n e) -> p n e", n=2)
                        n0 = blk * NSUB + h * 2
                        nc.gpsimd.dma_start(xin3, x_r[:, n0:n0 + 2, :])
                        xins.append(xin3)

                # ---- transpose x -> x^T (bf16): 2 e-chunks per PSUM bank ----
                xts = []
                for j in range(ECH // 2):
                    tp = ps_t.tile([128, 2 * TBLK], BF16, tag="ps_t")
                    for half in range(2):
                        c = 2 * j + half
                        for s in range(NSUB):
                            nc.tensor.transpose(
                                tp[:, half * TBLK + s * 128:
                                   half * TBLK + (s + 1) * 128],
                                xins[s // 2][:, s % 2, c * 128:(c + 1) * 128],
                                ident16[:],
                            )
                    xt_j = xtp.tile([128, 2 * TBLK], BF16)
                    if j % 2 == 0:
                        nc.vector.tensor_copy(xt_j[:], tp[:])
                    else:
                        nc.scalar.activation(xt_j[:], tp[:], AF.Copy)
                    xts.extend([xt_j[:, :TBLK], xt_j[:, TBLK:]])
                    if j == 1 and pending_retr:
                        for qkT_p, blk_p in pending_retr:
                            emit_retranspose_r(qkT_p, blk_p, blk_p == 0, False)
                        pending_retr.clear()

                pending_pass.append((xts, blk))
                if blk in (0, 1, 2, 4, 6, 7):
                    # clustered pass matmuls: dense real-MM burst warms the PE
                    for xts_p, blk_p in pending_pass:
                        emit_passes(xts_p, blk_p)
                    pending_pass.clear()

            pending_retr.reverse()
            for i, (qkT_p, blk_p) in enumerate(pending_retr):
                emit_retranspose_r(qkT_p, blk_p, False,
                                   i == len(pending_retr) - 1)
            pending_retr.clear()

            # ---- softmax on R (64x64): fused mask-add from PSUM ----
            r_sb = smallp.tile([64, 64], F32)
            nc.vector.tensor_add(r_sb[:], ps_R[:], mask_sb[:])
            negmax = smallp.tile([64, 1], F32)
            nc.vector.reduce_max(negmax[:], r_sb[:], axis=AX.X, negate=True)
            p_exp = smallp.tile([64, 64], F32)
            rowsum = smallp.tile([64, 1], F32)
            nc.scalar.activation(p_exp[:], r_sb[:], AF.Exp,
                                 bias=negmax[:], scale=1.0, accum_out=rowsum[:])
            rinv = smallp.tile([64, 1], F32)
            nc.vector.reciprocal(rinv[:], rowsum[:])
            p_r = smallp.tile([64, 64], BF16)
            nc.vector.tensor_scalar_mul(p_r[:], p_exp[:], rinv[:])

            # ---- O = V @ P : lhsT = V^T chunks, rhs = P; DMA out per group ----
            for g in range(4):
                po = ps_t.tile([128, 512], F32, tag="ps_t")
                for k in range(8):
                    c = g * 8 + k
                    nc.tensor.matmul(
                        po[:, k * D:(k + 1) * D],
                        vT[:, c * 128:(c + 1) * 128], p_r[:],
                        start=True, stop=True,
                    )
                o_sb = osbp.tile([128, 512], F32)
                if g % 2 == 0:
                    nc.scalar.activation(o_sb[:], po[:], AF.Copy)
                else:
                    nc.vector.tensor_copy(o_sb[:], po[:])
                nc.sync.dma_start(
                    out_r[:, g * 8:(g + 1) * 8, :],
                    o_sb[:].rearrange("p (c d) -> p c d", c=8),
                )

    nc.compile()
    return nc


def kernel(x, Wq, Wk, Wv):
    global _COMPILED
    if _COMPILED is None:
        _COMPILED = _build()
    nc = _COMPILED

    x = np.ascontiguousarray(np.asarray(x), dtype=np.float32)
    # fold the 1/sqrt(E) score scale into Wq (1/32 is exact in f32)
    wqk_h = np.ascontiguousarray(
        np.concatenate([np.asarray(Wq) * (1.0 / 32.0), np.asarray(Wk)], axis=1),
        dtype=np.float32,
    )
    wv_np = np.asarray(Wv)
    wv_h = np.ascontiguousarray(
        np.concatenate([wv_np, wv_np], axis=1), dtype=np.float32)

    in_maps = [
        {"x": np.ascontiguousarray(x[b]), "wqk": wqk_h, "wv": wv_h}
        for b in range(B)
    ]
    res = run_bass_kernel_spmd(nc, in_maps, list(range(N_CORES)))
    return np.stack([res.results[b]["out"] for b in range(B)], axis=0)

